# revision 15
# baseline (speedup 1.0000x reference)
"""Trainium2 Bass kernel for nn_AdaptiveEmbeddingI2T.

Computes, for image-batch shard i on each of 8 NeuronCores:
  sims[i, b] = <img_vec_i, txt_vec_ib> with
  txt_vec_ib = l2norm_d( mean_t( softmax_t(10*(gam_id*xn_bdt+bet_id)) * (gam*xn+bet) ) )

Device-side algebra (softmax shift-invariance folds BN into an affine):
  exponent = es*cap + eb with es = 10*gam*rs, eb = -es*mu
  txt_vec = wscale*(S2/S1) + wbias, S1 = sum_t e, S2 = sum_t e*cap
  sims = (sum_d v*w) * rsqrt(sum_d w^2) * rsqrt(sum_d v^2)

Layout: cap arrives pre-transposed/cast from host as bf16 [d%128, (dt,t,b)]
(t-major columns). Per (i,dt) the main loop computes e (ACT Exp) and
q=e*cap (DVE) side by side in [e|q] 128-col slabs, then a contiguous
slab-halving add tree (first level split DVE/GPSIMD) yields [S1|S2].
BN stats: sum on DVE tensor_tensor_reduce, sum-of-squares on ACT accum.

Sharding: image batch axis across 8 cores (8 images/core); cap + params
replicated; host concatenates the (8, 64) row blocks.
"""

import sys

if "/opt/trn_rl_repo" not in sys.path:
    sys.path.insert(0, "/opt/trn_rl_repo")

import numpy as np

import concourse.bacc as bacc
import concourse.mybir as mybir
from concourse.bass_utils import run_bass_kernel_spmd
from concourse.tile import TileContext

B_IMG, B_CAP, T_CAP, D = 64, 64, 64, 1024
H = 128
T_IMG = 36
EPS = 1e-5
N_CORES = 8
BI = B_IMG // N_CORES          # images per core
R = B_CAP * T_IMG              # 2304 caption cols per dt (t-major: col = t*64+b)
NDT = D // 128                 # 8 channel tiles

F32 = mybir.dt.float32
BF16 = mybir.dt.bfloat16

_COMPILED = None


def _build():
    nc = bacc.Bacc("TRN2", target_bir_lowering=False, debug=False,
                   num_devices=N_CORES)
    tensors = _declare_io(nc)
    with TileContext(nc) as tc:
        _emit(nc, tc, *tensors)
    nc.compile()
    return nc


def _emit(nc, tc, capT_d, img_d, wg1_d, wg2_d, wb1_d, wb2_d,
          bg1_d, bb1_d, bg2p1_d, bb2_d, o36_d, out_d, reps_main=1):
    AF = mybir.ActivationFunctionType
    ALU = mybir.AluOpType
    AX = mybir.AxisListType
    import os
    import contextlib
    _stage = int(os.environ.get("KSTAGE", "7"))
    _gs = int(os.environ.get("KGS", "1536"))      # L1 cols on gpsimd
    _wb = int(os.environ.get("KWBUFS", "4"))
    ctx = contextlib.ExitStack()
    with ctx:
        const = ctx.enter_context(tc.tile_pool(name="const", bufs=1))
        stream = ctx.enter_context(tc.tile_pool(name="stream", bufs=2))
        imgs = ctx.enter_context(tc.tile_pool(name="imgs", bufs=2))
        work = ctx.enter_context(tc.tile_pool(name="work", bufs=_wb))
        small = ctx.enter_context(tc.tile_pool(name="small", bufs=1))
        actx = ctx.enter_context(contextlib.ExitStack())
        ppool = actx.enter_context(tc.tile_pool(name="psum", bufs=2, space="PSUM"))
        pacc = actx.enter_context(tc.tile_pool(name="pacc", bufs=1, space="PSUM"))

        # ---- cap first: DMA per dt chunk, stats pipelined behind chunks ----
        capTb = const.tile([128, NDT * R], BF16, tag="capTb")  # 4.6 MB
        capT3 = capTb[:].rearrange("p (c r) -> p c r", c=NDT)
        capTd3 = capT_d[:].rearrange("p (c r) -> p c r", c=NDT)
        for dt in range(NDT):
            nc.sync.dma_start(out=capT3[:, dt, :], in_=capTd3[:, dt, :])

        # BN stats per channel, split across DVE (TTR) and ACT (Square+accum)
        _sdve = int(os.environ.get("KSDVE", "3"))   # dts whose sum runs on DVE
        _ttr_on = os.environ.get("KTTR", "0") == "1"  # TTR crashes real HW
        musum = small.tile([128, NDT], F32, tag="musum")
        sqsum = small.tile([128, NDT], F32, tag="sqsum")
        for dt in range(NDT):
            scr = stream.tile([128, R], BF16, tag="scr")
            if dt < _sdve and _ttr_on:
                nc.vector.tensor_tensor_reduce(
                    scr[:], capT3[:, dt, :], capT3[:, dt, :], 1.0, 0.0,
                    op0=ALU.max, op1=ALU.add,
                    accum_out=musum[:, dt:dt + 1])
            else:
                nc.scalar.activation(scr[:], capT3[:, dt, :], AF.Copy,
                                     accum_out=musum[:, dt:dt + 1])
            scr2 = stream.tile([128, R], BF16, tag="scr2")
            if dt < NDT - _sdve or not _ttr_on:
                nc.scalar.activation(scr2[:], capT3[:, dt, :], AF.Square,
                                     accum_out=sqsum[:, dt:dt + 1])
            else:
                nc.vector.tensor_tensor_reduce(
                    scr2[:], capT3[:, dt, :], capT3[:, dt, :], 1.0, 0.0,
                    op0=ALU.mult, op1=ALU.add,
                    accum_out=sqsum[:, dt:dt + 1])

        # ---- small params ----
        bg1_s = const.tile([H, 1], F32, tag="bg1")
        nc.sync.dma_start(out=bg1_s[:], in_=bg1_d[:])
        bb1_s = const.tile([H, 1], F32, tag="bb1")
        nc.sync.dma_start(out=bb1_s[:], in_=bb1_d[:])
        bg2p1_s = const.tile([128, NDT], F32, tag="bg2p1")
        nc.sync.dma_start(out=bg2p1_s[:], in_=bg2p1_d[:])
        bb2_s = const.tile([128, NDT], F32, tag="bb2t")
        nc.sync.dma_start(out=bb2_s[:], in_=bb2_d[:])
        o36_s = const.tile([T_IMG, 1], F32, tag="o36")
        nc.sync.dma_start(out=o36_s[:], in_=o36_d[:])

        # ---- MLP weights: stream f32 -> resident bf16 ----
        wg1_b = const.tile([128, NDT * H], BF16, tag="wg1b")
        wb1_b = const.tile([128, NDT * H], BF16, tag="wb1b")
        wg2_b = const.tile([128, D], BF16, tag="wg2b")
        wb2_b = const.tile([128, D], BF16, tag="wb2b")
        for w_d, w_b, view in ((wg1_d, wg1_b, True), (wb1_d, wb1_b, True),
                               (wg2_d, wg2_b, False), (wb2_d, wb2_b, False)):
            ws = stream.tile([128, D], F32, tag="stream")
            if view:
                nc.sync.dma_start(
                    out=ws[:].rearrange("p (c h) -> p c h", c=NDT),
                    in_=w_d[:].rearrange("(c p) h -> p c h", p=128))
            else:
                nc.sync.dma_start(out=ws[:], in_=w_d[:])
            nc.vector.tensor_copy(w_b[:], ws[:])

        ones_b = const.tile([128, 1], BF16, tag="onesb")
        nc.gpsimd.memset(ones_b[:], 1.0)

        if _stage < 2:
            res = small.tile([1, BI * B_CAP], F32, tag="res")
            nc.vector.tensor_copy(res[:], capT3[0:1, 0, 0:BI * B_CAP])
            nc.sync.dma_start(out=out_d[:].rearrange("i b -> (i b)"), in_=res[:])
            return
        # ---- BN stats finalize ----
        mu = small.tile([128, NDT], F32, tag="mu")
        rs = small.tile([128, NDT], F32, tag="rs")
        tv = small.tile([128, NDT], F32, tag="tv")
        # var = E[x^2] - mu^2 ; rs = 1/sqrt(var+eps)
        nc.vector.tensor_scalar_mul(mu[:], musum[:], 1.0 / R)
        nc.vector.tensor_tensor(tv[:], mu[:], mu[:], op=ALU.mult)
        nc.vector.tensor_scalar(sqsum[:], sqsum[:], 1.0 / R, None,
                                op0=ALU.mult)
        nc.vector.tensor_tensor(tv[:], sqsum[:], tv[:], op=ALU.subtract)
        nc.vector.tensor_scalar_add(tv[:], tv[:], EPS)
        nc.scalar.sqrt(tv[:], tv[:])
        nc.vector.reciprocal(rs[:], tv[:])

        if _stage < 4:
            res = small.tile([1, BI * B_CAP], F32, tag="res")
            nc.vector.tensor_copy(res[:, 0:NDT], mu[0:1, :])
            nc.sync.dma_start(out=out_d[:].rearrange("i b -> (i b)"), in_=res[:])
            return
        # ---- image means, directly transposed: imgrT [128, (dt i)] ----
        imgrT_ps = pacc.tile([128, NDT * BI], F32, tag="imgrT_ps")
        for i in range(BI):
            ichunk = imgs.tile([T_IMG, D], F32, tag="ichunk")
            nc.sync.dma_start(out=ichunk[:], in_=img_d[i])
            for dt in range(NDT):
                nc.tensor.matmul(
                    imgrT_ps[:, dt * BI + i:dt * BI + i + 1],
                    lhsT=ichunk[:, dt * 128:(dt + 1) * 128], rhs=o36_s[:],
                    start=True, stop=True)

        imgrT = const.tile([128, NDT * BI], F32, tag="imgrT")
        imgrTb = const.tile([128, NDT * BI], BF16, tag="imgrTb")
        imgrT3 = imgrT[:].rearrange("p (c i) -> p c i", c=NDT)
        imgrTb3 = imgrTb[:].rearrange("p (c i) -> p c i", c=NDT)
        nc.vector.tensor_copy(imgrT[:], imgrT_ps[:])
        nc.scalar.copy(imgrTb[:], imgrT_ps[:])

        # rnorm: 1/||v_i|| via accumulating [1,1] matmuls
        nrm2_ps = pacc.tile([1, BI], F32, tag="nrm2_ps")
        for i in range(BI):
            for dt in range(NDT):
                nc.tensor.matmul(
                    nrm2_ps[:, i:i + 1],
                    lhsT=imgrT3[:, dt, i:i + 1], rhs=imgrT3[:, dt, i:i + 1],
                    start=(dt == 0), stop=(dt == NDT - 1))
        nrm_row = small.tile([1, BI], F32, tag="nrm_row")
        nc.scalar.sqrt(nrm_row[:], nrm2_ps[:])
        rsr_row = small.tile([1, BI], F32, tag="rsr_row")
        nc.vector.reciprocal(rsr_row[:], nrm_row[:])

        if _stage < 5:
            res = small.tile([1, BI * B_CAP], F32, tag="res")
            nc.vector.tensor_copy(res[:, 0:BI], rsr_row[:])
            nc.sync.dma_start(out=out_d[:].rearrange("i b -> (i b)"), in_=res[:])
            return
        # ---- CBN MLPs -> per-(d,i) scales/biases ----
        wg1_b3 = wg1_b[:].rearrange("p (c h) -> p c h", c=NDT)
        wb1_b3 = wb1_b[:].rearrange("p (c h) -> p c h", c=NDT)

        def mlp_head(w1_b3, b1_s, w2_b, b2_s, name):
            h_ps = ppool.tile([H, BI], F32, tag="tr")
            for dt in range(NDT):
                nc.tensor.matmul(h_ps[:], lhsT=w1_b3[:, dt, :],
                                 rhs=imgrTb3[:, dt, :],
                                 start=(dt == 0), stop=(dt == NDT - 1))
            hT = small.tile([H, BI], BF16, tag=f"hT_{name}")
            nc.scalar.activation(hT[:], h_ps[:], AF.Relu, bias=b1_s[:], scale=1.0)
            outT = const.tile([128, NDT * BI], F32, tag=f"outT_{name}")
            outT3 = outT[:].rearrange("p (c i) -> p c i", c=NDT)
            for dt in range(NDT):
                o_ps = ppool.tile([128, BI], F32, tag="tr")
                nc.tensor.matmul(o_ps[:], lhsT=w2_b[:, dt * 128:(dt + 1) * 128],
                                 rhs=hT[:], start=True, stop=True)
                nc.scalar.activation(outT3[:, dt, :], o_ps[:], AF.Identity,
                                     bias=b2_s[:, dt:dt + 1], scale=1.0)
            return outT3

        gamT3 = mlp_head(wg1_b3, bg1_s, wg2_b, bg2p1_s, "g")
        betT3 = mlp_head(wb1_b3, bb1_s, wb2_b, bb2_s, "b")

        # escale = 10*gam*rs ; ebias = -escale*mu
        # wscale = gam*rs/36 ; wbias = (bet - gam*rs*mu)/36
        escale = const.tile([128, NDT * BI], F32, tag="escale")
        ebias = const.tile([128, NDT * BI], F32, tag="ebias")
        wscale = const.tile([128, NDT * BI], F32, tag="wscale")
        wbias = const.tile([128, NDT * BI], F32, tag="wbias")
        es3 = escale[:].rearrange("p (c i) -> p c i", c=NDT)
        eb3 = ebias[:].rearrange("p (c i) -> p c i", c=NDT)
        ws3 = wscale[:].rearrange("p (c i) -> p c i", c=NDT)
        wb3 = wbias[:].rearrange("p (c i) -> p c i", c=NDT)
        grs = small.tile([128, BI], F32, tag="grs")
        tmp = small.tile([128, BI], F32, tag="tmpb")
        negmu = small.tile([128, 1], F32, tag="negmu")
        for dt in range(NDT):
            nc.vector.tensor_scalar(grs[:], gamT3[:, dt, :], rs[:, dt:dt + 1],
                                    None, op0=ALU.mult)
            nc.vector.tensor_scalar_mul(es3[:, dt, :], grs[:], 10.0)
            nc.vector.tensor_scalar_mul(negmu[:], mu[:, dt:dt + 1], -1.0)
            nc.vector.tensor_scalar(eb3[:, dt, :], es3[:, dt, :], negmu[:],
                                    None, op0=ALU.mult)
            nc.vector.tensor_scalar_mul(ws3[:, dt, :], grs[:], 1.0 / 36.0)
            nc.vector.tensor_scalar(tmp[:], grs[:], mu[:, dt:dt + 1],
                                    None, op0=ALU.mult)
            nc.vector.tensor_tensor(tmp[:], betT3[:, dt, :], tmp[:],
                                    op=ALU.subtract)
            nc.vector.tensor_scalar_mul(wb3[:, dt, :], tmp[:], 1.0 / 36.0)

        if _stage < 7:
            res = small.tile([1, BI * B_CAP], F32, tag="res")
            nc.vector.tensor_copy(res[:, 0:BI], es3[0:1, 0, :])
            nc.sync.dma_start(out=out_d[:].rearrange("i b -> (i b)"), in_=res[:])
            return
        # ---- main loop over (i, dt) ----
        actx.close()  # release phase-A psum banks
        pmain = ctx.enter_context(tc.tile_pool(name="pmain", bufs=1,
                                               space="PSUM"))
        dot_ps = pmain.tile([1, BI * B_CAP], F32, tag="dot_ps")
        nrm_ps = pmain.tile([1, BI * B_CAP], F32, tag="nrm_ps")
        _lag = int(os.environ.get("KLAG", "2"))

        _gq = int(os.environ.get("KGQ", "384"))  # L1 q-cols on gpsimd
        _pool_on = os.environ.get("KPOOL", "1") == "1"

        def stage_a(i, dt):
            # eq: [e|q] slabs, col = t*128 + s*64 + b
            eq = work.tile([128, 2 * R], BF16, tag="eq")
            eqv = eq[:].rearrange("p (t s b) -> p t s b", t=T_IMG, s=2)
            ct_tb = capT3[:, dt, :].rearrange("p (t b) -> p t b", t=T_IMG)
            nc.scalar.activation(eqv[:, :, 0, :], ct_tb, AF.Exp,
                                 bias=eb3[:, dt, i:i + 1],
                                 scale=es3[:, dt, i:i + 1])
            # L1 e-half on gpsimd: depends only on ACT exp, starts early
            a1 = work.tile([128, 18 * 128], BF16, tag="a1")
            a1v = a1[:].rearrange("p (t s b) -> p t s b", t=18, s=2)
            eng0 = nc.gpsimd if _pool_on else nc.vector
            eng0.tensor_tensor(a1v[:, :, 0, :], eqv[:, 0:18, 0, :],
                               eqv[:, 18:36, 0, :], op=ALU.add)
            # q = e*cap on DVE
            nc.vector.tensor_tensor(eqv[:, :, 1, :], eqv[:, :, 0, :],
                                    ct_tb, op=ALU.mult)
            # L1 q-half split gpsimd/DVE
            eq1v = eqv[:, 0:18, 1, :]
            eq2v = eqv[:, 18:36, 1, :]
            a1q = a1v[:, :, 1, :]
            if _gq > 0 and _pool_on:
                gt = _gq // 64
                nc.gpsimd.tensor_tensor(a1q[:, 0:gt, :], eq1v[:, 0:gt, :],
                                        eq2v[:, 0:gt, :], op=ALU.add)
                nc.vector.tensor_tensor(a1q[:, gt:18, :], eq1v[:, gt:18, :],
                                        eq2v[:, gt:18, :], op=ALU.add)
            else:
                nc.vector.tensor_tensor(a1q, eq1v, eq2v, op=ALU.add)
            return i, dt, a1

        _gs2 = int(os.environ.get("KGS2", "0"))     # L2 cols on gpsimd

        def stage_b(state, s12all):
            i, dt, a1 = state
            # tree: 18 -> 9 -> (4+1) -> 2 -> 1 (+ slab 8)
            a2 = work.tile([128, 9 * 128], BF16, tag="a2")
            if _gs2 > 0:
                nc.gpsimd.tensor_tensor(a2[:, 0:_gs2], a1[:, 0:_gs2],
                                        a1[:, 1152:1152 + _gs2], op=ALU.add)
            if _gs2 < 1152:
                nc.vector.tensor_tensor(a2[:, _gs2:1152], a1[:, _gs2:1152],
                                        a1[:, 1152 + _gs2:2304], op=ALU.add)
            a3 = work.tile([128, 512], BF16, tag="a3")
            nc.vector.tensor_tensor(a3[:], a2[:, 0:512],
                                    a2[:, 512:1024], op=ALU.add)
            a4 = work.tile([128, 256], BF16, tag="a4")
            nc.vector.tensor_tensor(a4[:], a3[:, 0:256],
                                    a3[:, 256:512], op=ALU.add)
            a5 = work.tile([128, 128], BF16, tag="a5")
            nc.vector.tensor_tensor(a5[:], a4[:, 0:128],
                                    a4[:, 128:256], op=ALU.add)
            nc.vector.tensor_tensor(
                s12all[:, dt * 128:(dt + 1) * 128], a5[:],
                a2[:, 1024:1152], op=ALU.add)

        def stage_c(i, s12all):
            # batched over dt: r = 1/S1, sc = S2*r
            s12v = s12all[:].rearrange("p (c s b) -> p c s b", c=NDT, s=2)
            r1 = work.tile([128, NDT * B_CAP], F32, tag="r1")
            r1v = r1[:].rearrange("p (c b) -> p c b", c=NDT)
            nc.vector.reciprocal(r1v, s12v[:, :, 0, :])
            sc = work.tile([128, NDT * B_CAP], F32, tag="sc")
            scv = sc[:].rearrange("p (c b) -> p c b", c=NDT)
            nc.vector.tensor_tensor(scv, s12v[:, :, 1, :], r1v, op=ALU.mult)
            for dt in range(NDT):
                w_t = work.tile([128, B_CAP], BF16, tag="w")
                nc.scalar.activation(w_t[:], scv[:, dt, :], AF.Identity,
                                     bias=wb3[:, dt, i:i + 1],
                                     scale=ws3[:, dt, i:i + 1])
                w2_t = work.tile([128, B_CAP], BF16, tag="w2")
                nc.scalar.square(w2_t[:], w_t[:])
                nc.tensor.matmul(dot_ps[:, i * B_CAP:(i + 1) * B_CAP],
                                 lhsT=imgrTb3[:, dt, i:i + 1], rhs=w_t[:],
                                 start=(dt == 0), stop=(dt == NDT - 1))
                nc.tensor.matmul(nrm_ps[:, i * B_CAP:(i + 1) * B_CAP],
                                 lhsT=ones_b[:], rhs=w2_t[:],
                                 start=(dt == 0), stop=(dt == NDT - 1))

        for _rep in range(reps_main):
            pending = []
            s12_by_i = {}
            done_by_i = {}

            def flush_one():
                state = pending.pop(0)
                fi = state[0]
                stage_b(state, s12_by_i[fi])
                done_by_i[fi] = done_by_i.get(fi, 0) + 1
                if done_by_i[fi] == NDT:
                    stage_c(fi, s12_by_i.pop(fi))

            for i in range(BI):
                s12_by_i[i] = work.tile([128, NDT * 128], F32, tag="s12all",
                                        bufs=2, name=f"s12all_{i}")
                for dt in range(NDT):
                    pending.append(stage_a(i, dt))
                    if len(pending) > _lag:
                        flush_one()
            while pending:
                flush_one()

            # ---- epilogue: sims = dot * rsqrt(nrm) * (1/|v|) ----
            rr = small.tile([1, BI * B_CAP], F32, tag="rr")
            nc.vector.reciprocal(rr[:], nrm_ps[:])
            rsn = small.tile([1, BI * B_CAP], F32, tag="rsn")
            nc.scalar.sqrt(rsn[:], rr[:])
            prod = small.tile([1, BI * B_CAP], F32, tag="prod")
            nc.vector.tensor_tensor(prod[:], dot_ps[:], rsn[:], op=ALU.mult)
            res = small.tile([1, BI * B_CAP], F32, tag="res")
            rsr_b = rsr_row[:].rearrange("p (i u) -> p i u", u=1).broadcast_to([1, BI, B_CAP])
            nc.vector.tensor_tensor(
                res[:].rearrange("p (i b) -> p i b", i=BI),
                prod[:].rearrange("p (i b) -> p i b", i=BI),
                rsr_b, op=ALU.mult)
            nc.sync.dma_start(out=out_d[:].rearrange("i b -> (i b)"), in_=res[:])


def _get_compiled():
    global _COMPILED
    if _COMPILED is None:
        _COMPILED = _build()
    return _COMPILED


def _declare_io(nc):
    return (
        nc.dram_tensor("capT", [128, NDT * R], BF16, kind="ExternalInput"),
        nc.dram_tensor("img", [BI, T_IMG, D], F32, kind="ExternalInput"),
        nc.dram_tensor("wg1", [D, H], F32, kind="ExternalInput"),
        nc.dram_tensor("wg2", [H, D], F32, kind="ExternalInput"),
        nc.dram_tensor("wb1", [D, H], F32, kind="ExternalInput"),
        nc.dram_tensor("wb2", [H, D], F32, kind="ExternalInput"),
        nc.dram_tensor("bg1", [H, 1], F32, kind="ExternalInput"),
        nc.dram_tensor("bb1", [H, 1], F32, kind="ExternalInput"),
        nc.dram_tensor("bg2p1", [128, NDT], F32, kind="ExternalInput"),
        nc.dram_tensor("bb2t", [128, NDT], F32, kind="ExternalInput"),
        nc.dram_tensor("o36", [T_IMG, 1], F32, kind="ExternalInput"),
        nc.dram_tensor("out", [BI, B_CAP], F32, kind="ExternalOutput"),
    )


def _build_repeated(reps):
    """Timing variant: run the compute `reps` times in one NEFF. With
    KREPMODE=main, phase A runs once and only the main loop repeats."""
    import os
    nc = bacc.Bacc("TRN2", target_bir_lowering=False, debug=False,
                   num_devices=N_CORES)
    tensors = _declare_io(nc)
    with TileContext(nc) as tc:
        if os.environ.get("KREPMODE") == "main":
            _emit(nc, tc, *tensors, reps_main=reps)
        else:
            for _ in range(reps):
                _emit(nc, tc, *tensors)
    nc.compile()
    return nc


def _in_maps(img_embed, cap_embed, Wg1, bg1, Wg2, bg2, Wb1, bb1, Wb2, bb2):
    import ml_dtypes
    # capT[p, dt, t, b] = cap[b, t, dt*128+p], t-major cols, bf16
    cap = np.asarray(cap_embed[:, :T_IMG, :], np.float32)       # [b, t, d]
    capT = cap.reshape(B_CAP, T_IMG, NDT, 128).transpose(3, 2, 1, 0)
    capT = np.ascontiguousarray(capT.reshape(128, NDT * R)).astype(
        ml_dtypes.bfloat16)
    shared = {
        "capT": capT,
        "wg1": np.ascontiguousarray(Wg1, np.float32),
        "wg2": np.ascontiguousarray(Wg2, np.float32),
        "wb1": np.ascontiguousarray(Wb1, np.float32),
        "wb2": np.ascontiguousarray(Wb2, np.float32),
        "bg1": np.ascontiguousarray(bg1.reshape(H, 1), np.float32),
        "bb1": np.ascontiguousarray(bb1.reshape(H, 1), np.float32),
        "bg2p1": np.ascontiguousarray((bg2 + 1.0).reshape(NDT, 128).T,
                                      np.float32),
        "bb2t": np.ascontiguousarray(bb2.reshape(NDT, 128).T, np.float32),
        "o36": np.full((T_IMG, 1), 1.0 / T_IMG, np.float32),
    }
    maps = []
    for c in range(N_CORES):
        m = dict(shared)
        m["img"] = np.ascontiguousarray(
            img_embed[c * BI:(c + 1) * BI], np.float32)
        maps.append(m)
    return maps


def kernel(img_embed, cap_embed, lens, Wg1, bg1, Wg2, bg2, Wb1, bb1, Wb2, bb2):
    del lens  # unused by the reference computation
    nc = _get_compiled()
    maps = _in_maps(np.asarray(img_embed), np.asarray(cap_embed),
                    np.asarray(Wg1), np.asarray(bg1), np.asarray(Wg2),
                    np.asarray(bg2), np.asarray(Wb1), np.asarray(bb1),
                    np.asarray(Wb2), np.asarray(bb2))
    import time as _time
    last = None
    for attempt in range(5):  # device occasionally needs runs to recover
        try:
            res = run_bass_kernel_spmd(nc, maps, core_ids=list(range(N_CORES)))
            break
        except Exception as e:
            last = e
            _time.sleep(10)
    else:
        raise last
    return np.concatenate([res.results[c]["out"] for c in range(N_CORES)],
                          axis=0).astype(np.float32)
